# revision 21
# baseline (speedup 1.0000x reference)
"""GCN layer (PyG GCNConv semantics) on 8 Trainium2 NeuronCores.

out = D^{-1/2} (A + I) D^{-1/2} (x @ W) + b

Strategy (graph/data parallel, destinations sharded across cores):
  - Factor the symmetric norm: out = dinv_dst * ((A+I)^T @ (dinv_src * (x@W))).
    dinv_src is folded into x on the host (pure per-row rescale of an input =
    layout prep); dinv_dst postscale runs on device.
  - Every core computes the full h' = (x*dinv) @ W with TensorE from a
    host-transposed bf16 x^T kept resident in SBUF.
  - Each core owns a 1250-destination slice. The host re-encodes its edge
    bucket as a dense count matrix A_c [10112 src, 1250 dst] in fp8e4
    (counts <=16 exact), streamed in 14 transfers at line rate.
  - Phase B contracts acc^T[f,dst] += h'^T @ A with fp8 DoubleRow matmuls
    (2 src tiles per instruction, 2x bf16 throughput): h' is quantized to
    fp8e4 (h8) on ACT, and a residual delta8 = h' - h8 (DVE) is accumulated
    for the first S_CORR src tiles, which bounds the quantization error of
    the whole pipeline at ~1.7e-2 relative (vs the 2e-2 gate; the exact
    value is verified offline against the reference pipeline).
  - The bias lands as a rank-1 K=1 matmul folded into each PSUM
    accumulation; postscale by dinv[dst] is a single DVE op per 512-column
    chunk, stored bf16 chunk-by-chunk; host reassembles.
"""

import sys

for _p in ("/opt/trn_rl_repo", "/root/.axon_site/_ro/trn_rl_repo"):
    if _p not in sys.path:
        sys.path.append(_p)

import numpy as np
import ml_dtypes

N_NODES = 10000
N_CORES = 8
PER_CORE = 1250  # dst nodes per core
D = 128
NPAD = 10240  # padded node count (80 tiles of 128)
DSTPAD = 1250  # per-core dst count
CHUNKS = [(0, 512), (512, 1024), (1024, 1250)]
NTB = 79  # source tiles streamed in phase B (tile 79 is all padding)
APAD = NTB * 128  # 10112 rows of A
DMAG = [2, 2, 4, 8, 8, 8, 8, 8, 8, 8, 8, 4, 3]  # A-stream transfers (tiles)
assert sum(DMAG) == NTB
S_CORR = 52  # src tiles 0..S_CORR-1 get the fp8 residual correction

_cache = {}


def _build_program(reps=1, a_dtype="float8e4", s_corr=S_CORR, stage=4,
                   unroll=8, staggered=False):
    """Build + finalize the SPMD Bass program (shape-independent).

    reps > 1 wraps the computation in a device-side For_i loop with the body
    unrolled 8x (reps % 8 == 0), so consecutive reps overlap across the
    loop's all-engine barrier. Timing: the axon RPC wall-clock floor is
    ~100ms, so K iterations on-device make the kernel time measurable as a
    slope."""
    import concourse.bacc as bacc
    import concourse.mybir as mybir
    import concourse.tile as tile

    nc = bacc.Bacc(None)
    bf16 = mybir.dt.bfloat16
    f32 = mybir.dt.float32
    adt = getattr(mybir.dt, a_dtype)

    xT_p = nc.declare_dram_parameter("xT", [128, NPAD], bf16, isOutput=False)
    w_p = nc.declare_dram_parameter("W", [128, 128], bf16, isOutput=False)
    dinvw_p = nc.declare_dram_parameter("dinvw", [1, DSTPAD], f32, isOutput=False)
    biasr_p = nc.declare_dram_parameter("biasr", [1, 128], bf16, isOutput=False)
    a_p = nc.declare_dram_parameter("A", [APAD, DSTPAD], adt, isOutput=False)
    out_p = nc.declare_dram_parameter("out", [128, DSTPAD], bf16, isOutput=True)

    with tile.TileContext(nc) as tc:
        with (
            tc.tile_pool(name="persist", bufs=1) as pp,
            tc.tile_pool(name="hps", bufs=4, space="PSUM") as hps,
            tc.tile_pool(name="aps", bufs=1, space="PSUM") as aps,
            tc.tile_pool(name="h8p", bufs=2) as h8p,
            tc.tile_pool(name="outp", bufs=2) as outp,
            tc.tile_pool(name="ap_sb", bufs=5) as ap_sb,
        ):
            w_sb = pp.tile([128, 128], bf16)
            nc.sync.dma_start(w_sb[:], w_p[:])
            dinvw1 = pp.tile([1, DSTPAD], f32)
            nc.sync.dma_start(dinvw1[:], dinvw_p[:])
            biasr = pp.tile([1, 128], bf16)
            nc.sync.dma_start(biasr[:], biasr_p[:])
            dinvw = pp.tile([128, DSTPAD], f32)
            nc.gpsimd.partition_broadcast(dinvw[:], dinvw1[:])
            ones = pp.tile([1, 512], bf16)
            nc.vector.memset(ones[:], 1.0)
            xT = pp.tile([128, NPAD], bf16)
            for i in range(4):
                sl = slice(i * NPAD // 4, (i + 1) * NPAD // 4)
                nc.sync.dma_start(xT[:, sl], xT_p[:, sl])

            args = (nc, mybir, adt, xT, w_sb, dinvw, biasr, ones,
                    a_p, out_p, hps, aps, h8p, outp, ap_sb, s_corr, stage)
            if reps == 1:
                _emit_body(*args)
            else:
                assert reps % unroll == 0, "reps must divide the unroll factor"
                hints = (mybir.EngineType.PE, mybir.EngineType.SP,
                         mybir.EngineType.DVE, mybir.EngineType.Activation)
                with tc.For_i(0, reps // unroll, 1, hint_engines=hints,
                              staggered_reset=staggered):
                    for _ in range(unroll):
                        _emit_body(*args)

    nc.finalize()
    return nc


def _emit_body(nc, mybir, adt, xT, w_sb, dinvw, biasr, ones,
               a_p, out_p, hps, aps, h8p, outp, ap_sb, s_corr, stage=4):
    bf16 = mybir.dt.bfloat16
    f32 = mybir.dt.float32
    fp8 = adt == mybir.dt.float8e4
    NG = len(DMAG)
    t0s = np.concatenate([[0], np.cumsum(DMAG)[:-1]]).tolist()

    pa = []
    for c, (w0, w1) in enumerate(CHUNKS):
        pac = aps.tile([128, w1 - w0], f32, tag=f"pa{c}", name=f"pa{c}")
        pa.append(pac)
    # per-chunk matmul counters for start/stop flags (incl. the bias matmul)
    if fp8:
        npairs = sum((gl + 1) // 2 for gl in DMAG)
        ndpairs = sum(
            (min(t0s[g] + DMAG[g], s_corr) - t0s[g] + 1) // 2
            for g in range(NG) if t0s[g] < s_corr
        )
        nmm_chunk = 1 + npairs + ndpairs
    else:
        nmm_chunk = 1 + NTB
    mm_idx = [0, 0, 0]

    def dma_group(g):
        t0, gl = t0s[g], DMAG[g]
        at = ap_sb.tile([128, 8, DSTPAD], adt, tag="at", name="at")
        nc.sync.dma_start(
            at[:, :gl, :],
            a_p[t0 * 128 : (t0 + gl) * 128, :].rearrange("(g p) d -> p g d", p=128),
        )
        return at

    def mm_flags(c):
        i = mm_idx[c]
        mm_idx[c] += 1
        return {"start": i == 0, "stop": i == nmm_chunk - 1}

    # ---- A stream -------------------------------------------------------
    ats = [dma_group(0), dma_group(1), dma_group(2)]

    h8 = d8 = None
    if stage >= 3:
        h8 = h8p.tile([128, 80, 128], adt if fp8 else bf16, tag="h8", name="h8")
        if fp8 and s_corr > 0:
            d8 = h8p.tile([128, s_corr, 128], adt, tag="d8", name="d8")

    def phase_a(g):
        """h' tiles for A-group g (matmul on PE, quantize on ACT/DVE)."""
        t0, gl = t0s[g], DMAG[g]
        for k0 in range(t0, t0 + gl, 4):
            kl = min(4, t0 + gl - k0)
            ph = hps.tile([128, 512], f32, tag="ph", name="ph")
            for j in range(kl):
                nc.tensor.matmul(
                    out=ph[:, j * 128 : (j + 1) * 128],
                    lhsT=xT[:, (k0 + j) * 128 : (k0 + j + 1) * 128],
                    rhs=w_sb[:],
                    start=True,
                    stop=True,
                )
            nc.scalar.activation(
                h8[:, k0 : k0 + kl, :],
                ph[:, : kl * 128].rearrange("p (g f) -> p g f", f=128),
                mybir.ActivationFunctionType.Copy,
            )
            if fp8 and k0 < s_corr:
                nc.vector.tensor_tensor(
                    out=d8[:, k0 : k0 + kl, :],
                    in0=ph[:, : kl * 128].rearrange("p (g f) -> p g f", f=128),
                    in1=h8[:, k0 : k0 + kl, :],
                    op=mybir.AluOpType.subtract,
                )

    # ---- phase B: acc^T[f,dst] += h'^T @ A ------------------------------
    def phase_b(g, at):
        t0, gl = t0s[g], DMAG[g]
        if not fp8:
            for c, (w0, w1) in enumerate(CHUNKS):
                for i in range(gl):
                    nc.tensor.matmul(
                        out=pa[c][:], lhsT=h8[:, t0 + i, :],
                        rhs=at[:, i, w0:w1], **mm_flags(c),
                    )
            return
        ncorr = max(0, min(t0 + gl, s_corr) - t0)
        for c, (w0, w1) in enumerate(CHUNKS):
            i = 0
            while i < gl:
                pair = i + 1 < gl
                srcs = [h8] + ([d8] if i < ncorr else [])
                for src in srcs:
                    # both h8 and d8 are indexed by global tile id
                    lh = src[:, t0 + i : t0 + i + 2, :] if pair else \
                        src[:, t0 + i, :]
                    if pair:
                        nc.tensor.matmul(
                            out=pa[c][:], lhsT=lh, rhs=at[:, i : i + 2, w0:w1],
                            perf_mode=mybir.MatmulPerfMode.DoubleRow,
                            **mm_flags(c),
                        )
                    else:
                        nc.tensor.matmul(
                            out=pa[c][:], lhsT=lh, rhs=at[:, i, w0:w1],
                            **mm_flags(c),
                        )
                i += 2 if pair else 1

    # interleaved schedule: phase-A group g+1 is emitted before phase-B
    # group g so its quantize (ACT/DVE) overlaps B(g)'s matmuls; the A-DMA
    # stream runs 3 groups ahead. The bias seed mms sit right before B(0) so
    # the previous body's postscale read of pa has drained by then.
    if stage >= 3:
        phase_a(0)
        phase_a(1)
        for g in range(NG):
            if g + 3 < NG:
                ats.append(dma_group(g + 3))
            if g + 2 < NG:
                phase_a(g + 2)
            if g == 0:
                for c, (w0, w1) in enumerate(CHUNKS):
                    nc.tensor.matmul(
                        out=pa[c][:], lhsT=biasr[:], rhs=ones[:, : w1 - w0],
                        **mm_flags(c)
                    )
            phase_b(g, ats[g])
    else:
        for g in range(NG):
            if g + 3 < NG:
                ats.append(dma_group(g + 3))
    assert stage < 3 or mm_idx == [nmm_chunk] * 3, (mm_idx, nmm_chunk)

    # ---- postscale + store ----------------------------------------------
    outsb = outp.tile([128, DSTPAD], bf16, tag="outsb", name="outsb")
    if stage >= 4:
        for c, (w0, w1) in enumerate(CHUNKS):
            nc.vector.tensor_tensor(
                out=outsb[:, w0:w1],
                in0=pa[c][:],
                in1=dinvw[:, w0:w1],
                op=mybir.AluOpType.mult,
            )
            nc.sync.dma_start(out_p[:, w0:w1], outsb[:, w0:w1])
    else:
        nc.vector.memset(outsb[:], 0.0)
        nc.sync.dma_start(out_p[:], outsb[:])


def _prep_inputs(x, adj, W, b, a_dtype="float8e4"):
    """Host-side sharding/layout: per-core dense count matrix, casts,
    transposes, and folding of the source-side degree norm into x (a pure
    per-row rescale; the matmuls, aggregation, bias and dst-side norm run
    on device)."""
    bf = ml_dtypes.bfloat16
    src = np.asarray(adj[0], dtype=np.int64)
    dst = np.asarray(adj[1], dtype=np.int64)
    x = np.asarray(x, dtype=np.float32)
    W = np.asarray(W, dtype=np.float32)
    b = np.asarray(b, dtype=np.float32)
    n = x.shape[0]
    assert n == N_NODES and x.shape[1] == D

    # self-loops as ordinary edges
    loops = np.arange(n, dtype=np.int64)
    allsrc = np.concatenate([src, loops])
    alldst = np.concatenate([dst, loops])

    deg = np.bincount(alldst, minlength=n).astype(np.float32)  # includes loops
    dinv = np.where(deg > 0, 1.0 / np.sqrt(deg), 0.0).astype(np.float32)

    xpad = np.zeros((NPAD, D), dtype=np.float32)
    xpad[:n] = x * dinv[:, None]
    xT = np.ascontiguousarray(xpad.T).astype(bf)
    W16 = W.astype(bf)
    biasr = np.ascontiguousarray(b.reshape(1, D)).astype(bf)

    corea = alldst // PER_CORE
    loc = alldst - corea * PER_CORE
    in_maps = []
    for c in range(N_CORES):
        m = corea == c
        key = allsrc[m] * DSTPAD + loc[m]
        counts = np.bincount(key, minlength=APAD * DSTPAD)
        adt = np.dtype("float8_e4m3") if a_dtype == "float8e4" else bf
        A = counts.reshape(APAD, DSTPAD).astype(adt)
        in_maps.append(
            {
                "xT": xT,
                "W": W16,
                "dinvw": np.ascontiguousarray(
                    dinv[c * PER_CORE : c * PER_CORE + DSTPAD].reshape(1, DSTPAD)
                ),
                "biasr": biasr,
                "A": A,
            }
        )
    return in_maps


def kernel(x, adj, W, b):
    from concourse.bass_utils import run_bass_kernel_spmd

    # edge multiplicities up to 16 are exact in fp8e4; else use bf16
    dst = np.asarray(adj[1], dtype=np.int64)
    src = np.asarray(adj[0], dtype=np.int64)
    maxmult = int(np.bincount(src * np.int64(N_NODES) + dst).max())
    a_dtype = "float8e4" if maxmult + 1 <= 16 else "bfloat16"
    if a_dtype not in _cache:
        _cache[a_dtype] = _build_program(a_dtype=a_dtype)
    nc = _cache[a_dtype]
    in_maps = _prep_inputs(x, adj, W, b, a_dtype)
    res = run_bass_kernel_spmd(nc, in_maps, list(range(N_CORES)))
    out = np.empty((N_NODES, D), dtype=np.float32)
    for c in range(N_CORES):
        ot = res.results[c]["out"]  # [128, 1250] = out^T (bf16)
        out[c * PER_CORE : (c + 1) * PER_CORE] = ot.T[:PER_CORE].astype(np.float32)
    return out
